# revision 1
# baseline (speedup 1.0000x reference)
"""Varlen causal GQA attention (B=4, S=1024, HQ=32, HK=8, D=128, fp32)
on 8 Trainium2 NeuronCores.

Sharding: tensor-parallel over the 8 kv heads (GQA groups stay together):
core i gets kv head i and query heads [4i, 4i+4), all 4 sequences. No
collectives; gather = concat along the head axis on host.

Per-core kernel, per (seq b, head-pair hp) over the full 1024-query span:
  for each 128-key tile kt (live query cols [128*kt, 1024), split at 512):
    scores_T[k,q] = K_tile^T.T @ Q^T     (float32r matmul, one per head)
    P_T = exp(scale * scores_T)          (ScalarE; one strided (2,w) exp
                                          covers both heads, PSUM->fp16)
    P_T[:, :, :128] *= causal triangle   (DVE, shared (128,128) mask)
  per head, per 128-query block qi (two PSUM chains share a bank):
    O[q,:128|128] += P_T_slice.T @ [V|1] (fp16 matmul; col 128 = sum exp)
    O = O[:, :128] * 1/O[:, 128]         (DVE reciprocal + broadcast mul)
The kernel is softmax(ScalarE-exp)-roofline-bound: ~0.85ns/col + ~335ns
per exp instruction; PSUM's 8 banks cap exp batching at (2,512) tiles.
Q/K arrive host-pre-transposed to (d, token) layout; V as fp16.
"""

import numpy as np
import ml_dtypes

import concourse.bass as bass
import concourse.tile as tile
import concourse.mybir as mybir
from concourse import bacc
from concourse.bass_utils import run_bass_kernel_spmd

B, S, D = 4, 1024, 128
HQ, HK = 32, 8
G = HQ // HK          # query heads per kv head (= per core)
N_CORES = 8
SCALE = 1.0 / float(np.sqrt(D))
KTW = 128             # key-tile width (matmul stationary free dim)
KT = S // KTW         # key tiles per sequence
NQI = S // 128        # 128-query blocks per sequence
MMW = 512             # max matmul moving free dim

F32 = mybir.dt.float32
F32R = mybir.dt.float32r
FP16 = mybir.dt.float16


def _score_bins():
    """Bin-pack the ragged live score pieces (kt, half, c0, w) into
    512-col PSUM bank rows. Widths: six 512s + 2x384 + 2x256 + 2x128
    -> exactly 9 bins of <=512."""
    pieces = []
    for kt in range(KT):
        c0 = KTW * kt
        if c0 < MMW:
            pieces.append((kt, 0, c0, MMW - c0))
            pieces.append((kt, 1, MMW, MMW))
        else:
            pieces.append((kt, 1, c0, S - c0))
    pieces.sort(key=lambda p: -p[3])  # first-fit decreasing
    bins = []
    for p in pieces:
        for abin in bins:
            if sum(x[3] for x in abin) + p[3] <= MMW:
                abin.append(p)
                break
        else:
            bins.append([p])
    return bins


SCORE_BINS = _score_bins()
# un-packed baseline: one bin per piece (12 exps per seq/head-pair)
SCORE_PIECES_UNPACKED = [[p] for abin in _score_bins() for p in abin]


def build_nc(repeat: int = 1, qk_dtype=F32R, ablate: str = "",
             mask_on_pool: bool = False, psp_bufs: int = 3, po_bufs: int = 2,
             use_divide: bool = False, bin_pack: bool = True):
    """Build the single-core Bass program (SPMD across 8 cores).

    repeat > 1 wraps the body in a hardware loop — used only for timing
    (marginal wall time per iteration approximates HW kernel time).
    ablate: timing-only variants with reduced work (WRONG results):
      "pv" = halve PV chains; "qk" = even key tiles only; "dve" = skip
      mask/normalize.
    """
    nc = bacc.Bacc(None, target_bir_lowering=False, debug=False)

    qT = nc.dram_tensor("qT", [G, B, D, S], qk_dtype, kind="ExternalInput")
    kT = nc.dram_tensor("kT", [B, D, S], qk_dtype, kind="ExternalInput")
    v = nc.dram_tensor("v", [B, S, D], FP16, kind="ExternalInput")
    mk = nc.dram_tensor("mk", [D, KTW], FP16, kind="ExternalInput")
    o = nc.dram_tensor("o", [B * S, G, D], F32, kind="ExternalOutput")
    # (b, g, p, qi, d) view of the output for per-(b,h) stores
    o_r = o[:].rearrange("(b qi p) g d -> b g p qi d", b=B, qi=NQI, p=128)

    with tile.TileContext(nc) as tc:
        with (
            tc.tile_pool(name="cpool", bufs=1) as cpool,
            tc.tile_pool(name="kpool", bufs=2) as kpool,
            tc.tile_pool(name="vpool", bufs=2) as vpool,
            tc.tile_pool(name="qpool", bufs=4) as qpool,
            tc.tile_pool(name="ppool", bufs=28) as ppool,
            tc.tile_pool(name="opool", bufs=4) as opool,
            tc.tile_pool(name="rpool", bufs=8) as rpool,
            tc.tile_pool(name="psp", bufs=psp_bufs, space="PSUM") as psp,
            tc.tile_pool(name="ps_o", bufs=po_bufs, space="PSUM") as ps_o,
        ):
            # shared causal triangle: mask[kk, q] = 1 iff q >= kk
            mask_t = cpool.tile([128, KTW], FP16)
            nc.sync.dma_start(out=mask_t[:], in_=mk[:])

            def emit_scores_pair(q_ts, kt_t):
                """QK^T + exp + triangle mask for all 8 key tiles of TWO
                heads at once. The ragged live pieces of all key tiles are
                bin-packed into full 512-col PSUM bank rows so one strided
                (2-head, <=512-col) exp covers each packed bank pair — 9
                exps per (seq, head-pair) instead of 12.

                Returns p_tiles[kt] = (half0_entry_or_None, half1_entry);
                entry = (fp16 tile (128, 2, 512), lo) with local col =
                global q col - lo."""
                bins = SCORE_BINS if bin_pack else SCORE_PIECES_UNPACKED
                piece_map = {}
                for abin in bins:
                    ps = psp.tile([128, 2, MMW], F32, tag="ps", name="ps")
                    pt = ppool.tile([128, 2, MMW], FP16, tag="pt", name="pt")
                    off = 0
                    for (kt, half, c0p, w) in abin:
                        for hh in range(2):
                            nc.tensor.matmul(
                                ps[:, hh, off:off + w],
                                lhsT=kt_t[:, kt * KTW:(kt + 1) * KTW],
                                rhs=q_ts[hh][:, c0p:c0p + w],
                                start=True, stop=True,
                            )
                        piece_map[(kt, half)] = (pt, c0p - off)
                        off += w
                    nc.scalar.activation(
                        pt[:, :, 0:off], ps[:, :, 0:off],
                        mybir.ActivationFunctionType.Exp, scale=SCALE,
                    )
                    # triangle mask on each kt's causal-boundary 128 cols
                    mask_eng = nc.gpsimd if mask_on_pool else nc.vector
                    boff = 0
                    for (kt, half, c0p, w) in abin:
                        if c0p == KTW * kt and ablate != "dve":
                            for hh in range(2):
                                mask_eng.tensor_mul(
                                    pt[:, hh, boff:boff + KTW],
                                    pt[:, hh, boff:boff + KTW], mask_t[:])
                        boff += w
                return [
                    (piece_map.get((kt, 0)), piece_map[(kt, 1)])
                    for kt in range(KT)
                ]

            def emit_pv(st, hh):
                """Probs @ [V|1] for one head of a pair, then normalize."""
                b, h0, p_tiles, v_t, o_ts = st
                o_t = o_ts[hh]
                for qih in range(NQI // 2):
                    # two 128-query accumulation chains share one PSUM bank
                    po = ps_o.tile([128, 2, KTW + 1], F32, tag="po", name="po")
                    for q2 in range(2):
                        qi = qih * 2 + q2
                        kts = [kt for kt in range(qi + 1)
                               if p_tiles[kt] is not None]
                        if ablate == "pv":
                            kts = kts[:len(kts) // 2 + 1]
                        for kt in kts:
                            pt, lo = p_tiles[kt][0 if qi < MMW // KTW else 1]
                            nc.tensor.matmul(
                                po[:, q2, :],
                                lhsT=pt[:, hh, qi * KTW - lo:
                                        (qi + 1) * KTW - lo],
                                rhs=v_t[:, kt, :],
                                start=(kt == kts[0]),
                                stop=(kt == kts[-1]),
                            )
                    if ablate != "dve":
                        if use_divide:
                            nc.vector.tensor_tensor(
                                o_t[:, qih * 2:qih * 2 + 2, :],
                                po[:, :, 0:KTW],
                                po[:, :, KTW:KTW + 1].broadcast_to(
                                    [128, 2, KTW]),
                                mybir.AluOpType.divide,
                            )
                        else:
                            rec = rpool.tile([128, 2], F32, tag="rec",
                                             name="rec")
                            nc.vector.reciprocal(rec[:], po[:, :, KTW])
                            nc.vector.tensor_mul(
                                o_t[:, qih * 2:qih * 2 + 2, :],
                                po[:, :, 0:KTW],
                                rec[:, :, None].broadcast_to([128, 2, KTW]),
                            )
                    else:
                        nc.vector.tensor_copy(
                            o_t[:, qih * 2, :], po[:, 0, 0:KTW])
                nc.gpsimd.dma_start(out=o_r[b, h0 + hh], in_=o_t[:])

            def body(_iv=None):
                pending = None  # one-pair-deep software pipeline
                for b in range(B):
                    kt_t = kpool.tile([128, S], qk_dtype, tag="kt", name="kt_t")
                    # first key tile separately so the first matmul can
                    # start before the bulk load lands (shortens the ramp)
                    nc.sync.dma_start(out=kt_t[:, 0:KTW], in_=kT[b][:, 0:KTW])
                    nc.sync.dma_start(out=kt_t[:, KTW:S], in_=kT[b][:, KTW:S])
                    v_t = vpool.tile([128, KT, KTW + 1], FP16, tag="vt", name="v_t")
                    nc.sync.dma_start(
                        out=v_t[:, :, 0:KTW],
                        in_=v[b].rearrange("(kt p) d -> p kt d", p=128),
                    )
                    nc.vector.memset(v_t[:, :, KTW:KTW + 1], 1.0)
                    for hp in range(G // 2):
                        h0 = hp * 2
                        q_ts, o_ts = [], []
                        for hh in range(2):
                            q_t = qpool.tile([128, S], qk_dtype, tag="qt",
                                             name="q_t")
                            # gpsimd queue: overlaps with the kt/v loads
                            # on the sync queue at each (b, pair) ramp
                            nc.gpsimd.dma_start(out=q_t[:, 0:MMW],
                                                in_=qT[h0 + hh, b][:, 0:MMW])
                            nc.gpsimd.dma_start(out=q_t[:, MMW:S],
                                                in_=qT[h0 + hh, b][:, MMW:S])
                            q_ts.append(q_t)
                            o_ts.append(opool.tile([128, NQI, KTW], F32,
                                                   tag="ot", name="o_t"))
                        p_tiles = emit_scores_pair(q_ts, kt_t)
                        if pending is not None:
                            emit_pv(pending, 0)
                            emit_pv(pending, 1)
                        pending = (b, h0, p_tiles, v_t, o_ts)
                if pending is not None:
                    emit_pv(pending, 0)
                    emit_pv(pending, 1)

            if repeat == 1:
                body()
            else:
                with tc.For_i(0, repeat, 1) as iv:
                    body(iv)

    nc.compile()
    return nc


def _build_mask() -> np.ndarray:
    """Shared diagonal-block triangle: mask[kk, q] = 1 iff q >= kk."""
    kk = np.arange(128)[:, None]
    qq = np.arange(KTW)[None, :]
    return (qq >= kk).astype(np.float16)


def _core_inputs(q: np.ndarray, k: np.ndarray, v: np.ndarray,
                 qk_np=np.float32):
    """Slice + lay out per-core inputs. Host-side shard/layout step."""
    mask = _build_mask()
    q5 = q.reshape(B, S, HK, G, D)
    k4 = k.reshape(B, S, HK, D)
    v4 = v.reshape(B, S, HK, D)
    in_maps = []
    for c in range(N_CORES):
        qT = np.ascontiguousarray(
            q5[:, :, c, :, :].transpose(2, 0, 3, 1)).astype(qk_np)  # (G,B,D,S)
        kT = np.ascontiguousarray(
            k4[:, :, c, :].transpose(0, 2, 1)).astype(qk_np)        # (B,D,S)
        vb = np.ascontiguousarray(v4[:, :, c, :]).astype(np.float16)
        in_maps.append({"qT": qT, "kT": kT, "v": vb, "mk": mask})
    return in_maps


_NC_CACHE = {}


def kernel(q, k, v, cu_seqlens_q=None, cu_seqlens_k=None,
           max_seqlen_q=None, max_seqlen_k=None) -> np.ndarray:
    q = np.asarray(q, dtype=np.float32)
    k = np.asarray(k, dtype=np.float32)
    v = np.asarray(v, dtype=np.float32)
    assert q.shape == (B * S, HQ, D) and k.shape == (B * S, HK, D)

    if "nc" not in _NC_CACHE:
        _NC_CACHE["nc"] = build_nc(repeat=1)
    nc = _NC_CACHE["nc"]

    in_maps = _core_inputs(q, k, v)
    res = None
    for attempt in range(3):
        try:
            res = run_bass_kernel_spmd(nc, in_maps,
                                       core_ids=list(range(N_CORES)))
            break
        except Exception:
            # a wedged NeuronCore fails once and resets; retry clean
            if attempt == 2:
                raise
            import time as _time
            _time.sleep(2.0)

    out = np.empty((B * S, HQ, D), np.float32)
    for c in range(N_CORES):
        out[:, c * G:(c + 1) * G, :] = res.results[c]["o"]
    return out



# revision 4
# speedup vs baseline: 1.0047x; 1.0047x over previous
"""Varlen causal GQA attention (B=4, S=1024, HQ=32, HK=8, D=128, fp32)
on 8 Trainium2 NeuronCores.

Sharding: tensor-parallel over the 8 kv heads (GQA groups stay together):
core i gets kv head i and query heads [4i, 4i+4), all 4 sequences. No
collectives; gather = concat along the head axis on host.

v2 layout: q/k/v/o all fp16 on the wire (scores err ~1e-3 << 2e-2 gate),
one DMA per tensor per (seq | pair) with host-contiguous layouts.
Per (seq b, head-pair hp):
  scores^T[k,q] = K_kt^T.T @ Q^T per live piece, binned into 6 PSUM
  groups alternating a 4-bank (128,2,1024) and a 2-bank (128,2,512)
  tile so ScalarE runs 6 exps/pair (3x N=2048 + 3x N=1024) while the
  other tile fills. All 8 causal-diagonal 128-wide pieces are packed
  into the first 4-bank group so one DVE mul applies the triangle mask
  for both heads.
  PV: per 128-query block qi, chain over key tiles with P^T stationary
  and [V|1] moving (col 128 = sum exp); two chains share a PSUM bank;
  DVE reciprocal+mul normalizes into fp16 o_t; one store per (b,h).
"""

import numpy as np
import ml_dtypes

import concourse.bass as bass
import concourse.tile as tile
import concourse.mybir as mybir
from concourse import bacc
from concourse.bass_utils import run_bass_kernel_spmd

B, S, D = 4, 1024, 128
HQ, HK = 32, 8
G = HQ // HK          # query heads per kv head (= per core)
N_CORES = 8
SCALE = 1.0 / float(np.sqrt(D))
KTW = 128             # key-tile width
KT = S // KTW         # key tiles per sequence
NQI = S // 128        # 128-query blocks per sequence

F32 = mybir.dt.float32
FP16 = mybir.dt.float16

# Live score pieces (kt, global q col c0, width w, tile col off), all
# w <= 512 (matmul moving limit) and 512-half-aligned inside their PSUM
# tile. Groups alternate a 4-bank (2,1024) and a 2-bank (2,512) tile so
# ScalarE runs 3 N=2048 + 3 N=1024 exps per pair while the other fills.
DIAG = [(kt, kt * KTW, KTW, kt * KTW) for kt in range(KT)]
SCORE_GROUPS = [
    (True, DIAG),           # all causal-diagonal pieces; masked after exp
    (False, [(3, 512, 512, 0)]),
    (True, [(0, 512, 512, 0), (0, 128, 384, 512), (2, 384, 128, 896)]),
    (False, [(1, 512, 512, 0)]),
    (True, [(2, 512, 512, 0), (4, 640, 384, 512), (6, 896, 128, 896)]),
    (False, [(5, 768, 256, 0), (1, 256, 256, 256)]),
]


def build_nc(repeat: int = 1, ablate: str = ""):
    """Build the single-core Bass program (SPMD across 8 cores).

    repeat > 1 wraps the body in a hardware loop - used only for timing
    (marginal wall time per iteration approximates HW kernel time).
    ablate: timing-only variants with reduced work (WRONG results):
      "pv" = halve PV chains; "qk" = skip small score groups; "dve" =
      skip mask/normalize.
    """
    nc = bacc.Bacc(None, target_bir_lowering=False, debug=False)

    qT = nc.dram_tensor("qT", [G // 2, B, D, 2, S], FP16, kind="ExternalInput")
    kT = nc.dram_tensor("kT", [B, D, S], FP16, kind="ExternalInput")
    v = nc.dram_tensor("v", [B, 128, KT, D], FP16, kind="ExternalInput")
    mk = nc.dram_tensor("mk", [D, S], FP16, kind="ExternalInput")
    o = nc.dram_tensor("o", [B, G, 128, NQI, D], FP16, kind="ExternalOutput")

    with tile.TileContext(nc) as tc:
        with (
            tc.tile_pool(name="cpool", bufs=1) as cpool,
            tc.tile_pool(name="kpool", bufs=2) as kpool,
            tc.tile_pool(name="vpool", bufs=2) as vpool,
            tc.tile_pool(name="qpool", bufs=2) as qpool,
            tc.tile_pool(name="pbig", bufs=6) as pbig,
            tc.tile_pool(name="psml", bufs=6) as psml,
            tc.tile_pool(name="opool", bufs=4) as opool,
            tc.tile_pool(name="rpool", bufs=8) as rpool,
            tc.tile_pool(name="psp4", bufs=1, space="PSUM") as psp4,
            tc.tile_pool(name="psp2", bufs=1, space="PSUM") as psp2,
            tc.tile_pool(name="ps_o", bufs=2, space="PSUM") as ps_o,
        ):
            # diagonal triangle strip: mk[kk, kt*128+q] = 1 iff q >= kk
            mask_t = cpool.tile([128, S], FP16)
            nc.sync.dma_start(out=mask_t[:], in_=mk[:])

            def emit_scores_pair(q_t, kt_t):
                """QK^T + exp (+ triangle mask on the diag group) for both
                heads of a pair. Returns piece_map[(kt, qi)] = (pt, lo):
                fp16 tile with local col = global q col - lo."""
                piece_map = {}
                for gi, (is_big, pieces) in enumerate(SCORE_GROUPS):
                    if ablate == "qk" and not is_big:
                        continue
                    w_tile = 1024 if is_big else 512
                    pool = psp4 if is_big else psp2
                    ppool = pbig if is_big else psml
                    ps = pool.tile([128, 2, w_tile], F32, tag="ps", name="ps")
                    pt = ppool.tile([128, 2, w_tile], FP16, tag="pt", name="pt")
                    w_used = 0
                    for (kt, c0, w, off) in pieces:
                        for hh in range(2):
                            nc.tensor.matmul(
                                ps[:, hh, off:off + w],
                                lhsT=kt_t[:, kt * KTW:(kt + 1) * KTW],
                                rhs=q_t[:, hh, c0:c0 + w],
                                start=True, stop=True,
                            )
                        lo = c0 - off
                        for qi in range(c0 // KTW, (c0 + w) // KTW):
                            piece_map[(kt, qi)] = (pt, lo)
                        w_used = max(w_used, off + w)
                    nc.scalar.activation(
                        pt[:, :, 0:w_used], ps[:, :, 0:w_used],
                        mybir.ActivationFunctionType.Exp, scale=SCALE,
                    )
                    if gi == 0 and ablate != "dve":
                        # one mul masks all 8 triangles for both heads
                        nc.vector.tensor_mul(
                            pt[:], pt[:],
                            mask_t[:, None, :].broadcast_to([128, 2, S]))
                return piece_map

            def emit_pv(st, hh):
                """Probs @ [V|1] for one head of a pair, then normalize."""
                b, h0, piece_map, v_t, o_ts = st
                o_t = o_ts[hh]
                for qih in range(NQI // 2):
                    # two 128-query accumulation chains share one PSUM bank
                    po = ps_o.tile([128, 2, KTW + 1], F32, tag="po", name="po")
                    for q2 in range(2):
                        qi = qih * 2 + q2
                        kts = list(range(qi + 1))
                        if ablate == "pv":
                            kts = kts[:len(kts) // 2 + 1]
                        for kt in kts:
                            pt, lo = piece_map[(kt, qi)]
                            nc.tensor.matmul(
                                po[:, q2, :],
                                lhsT=pt[:, hh, qi * KTW - lo:
                                        (qi + 1) * KTW - lo],
                                rhs=v_t[:, kt, :],
                                start=(kt == kts[0]),
                                stop=(kt == kts[-1]),
                            )
                    if ablate != "dve":
                        rec = rpool.tile([128, 2], F32, tag="rec", name="rec")
                        nc.vector.reciprocal(rec[:], po[:, :, KTW])
                        nc.vector.tensor_mul(
                            o_t[:, qih * 2:qih * 2 + 2, :],
                            po[:, :, 0:KTW],
                            rec[:, :, None].broadcast_to([128, 2, KTW]),
                        )
                    else:
                        nc.vector.tensor_copy(
                            o_t[:, qih * 2, :], po[:, 0, 0:KTW])
                nc.gpsimd.dma_start(out=o[b, h0 + hh], in_=o_t[:])

            def body(_iv=None):
                pending = None  # one-pair-deep software pipeline
                for b in range(B):
                    kt_t = kpool.tile([128, S], FP16, tag="kt", name="kt_t")
                    nc.sync.dma_start(out=kt_t[:], in_=kT[b])
                    v_t = vpool.tile([128, KT, KTW + 1], FP16, tag="vt",
                                     name="v_t")
                    nc.sync.dma_start(out=v_t[:, :, 0:KTW], in_=v[b])
                    nc.vector.memset(v_t[:, :, KTW:KTW + 1], 1.0)
                    for hp in range(G // 2):
                        h0 = hp * 2
                        q_t = qpool.tile([128, 2, S], FP16, tag="qt",
                                         name="q_t")
                        # gpsimd queue: overlaps with kt/v on the sync queue
                        nc.gpsimd.dma_start(out=q_t[:], in_=qT[hp, b])
                        o_ts = [opool.tile([128, NQI, KTW], FP16, tag="ot",
                                           name="o_t") for _ in range(2)]
                        piece_map = emit_scores_pair(q_t, kt_t)
                        if pending is not None:
                            emit_pv(pending, 0)
                            emit_pv(pending, 1)
                        pending = (b, h0, piece_map, v_t, o_ts)
                if pending is not None:
                    emit_pv(pending, 0)
                    emit_pv(pending, 1)

            if repeat == 1:
                body()
            else:
                with tc.For_i(0, repeat, 1) as iv:
                    body(iv)

    nc.compile()
    return nc


def _build_mask() -> np.ndarray:
    """Diagonal-block triangle strip: mk[kk, kt*128+q] = 1 iff q >= kk."""
    kk = np.arange(128)[:, None]
    qq = np.arange(KTW)[None, :]
    tri = (qq >= kk).astype(np.float16)
    return np.tile(tri, (1, KT))


def _core_inputs(q: np.ndarray, k: np.ndarray, v: np.ndarray):
    """Slice + lay out per-core inputs. Host-side shard/layout step."""
    mask = _build_mask()
    q5 = q.reshape(B, S, HK, G, D)
    k4 = k.reshape(B, S, HK, D)
    v4 = v.reshape(B, S, HK, D)
    in_maps = []
    for c in range(N_CORES):
        qt = q5[:, :, c, :, :].transpose(2, 0, 3, 1)          # (G,B,D,S)
        qT = np.ascontiguousarray(
            qt.reshape(G // 2, 2, B, D, S).transpose(0, 2, 3, 1, 4)
        ).astype(np.float16)                                   # (G/2,B,D,2,S)
        kT = np.ascontiguousarray(
            k4[:, :, c, :].transpose(0, 2, 1)).astype(np.float16)  # (B,D,S)
        vb = np.ascontiguousarray(
            v4[:, :, c, :].reshape(B, KT, 128, D).transpose(0, 2, 1, 3)
        ).astype(np.float16)                                   # (B,128,KT,D)
        in_maps.append({"qT": qT, "kT": kT, "v": vb, "mk": mask})
    return in_maps


def _unshard(core_outs) -> np.ndarray:
    """core_outs[c]: (B, G, 128, NQI, D) fp16 -> (B*S, HQ, D) fp32."""
    out = np.empty((B, S, HQ, D), np.float32)
    for c, oc in enumerate(core_outs):
        # (B,G,128,NQI,D) -> (B, S=NQI*128, G, D)
        ob = np.asarray(oc, dtype=np.float32).transpose(0, 3, 2, 1, 4)
        out[:, :, c * G:(c + 1) * G, :] = ob.reshape(B, S, G, D)
    return out.reshape(B * S, HQ, D)


_NC_CACHE = {}


def kernel(q, k, v, cu_seqlens_q=None, cu_seqlens_k=None,
           max_seqlen_q=None, max_seqlen_k=None) -> np.ndarray:
    q = np.asarray(q, dtype=np.float32)
    k = np.asarray(k, dtype=np.float32)
    v = np.asarray(v, dtype=np.float32)
    assert q.shape == (B * S, HQ, D) and k.shape == (B * S, HK, D)

    if "nc" not in _NC_CACHE:
        _NC_CACHE["nc"] = build_nc(repeat=1)
    nc = _NC_CACHE["nc"]

    in_maps = _core_inputs(q, k, v)
    res = None
    for attempt in range(3):
        try:
            res = run_bass_kernel_spmd(nc, in_maps,
                                       core_ids=list(range(N_CORES)))
            break
        except Exception:
            # a wedged NeuronCore fails once and resets; retry clean
            if attempt == 2:
                raise
            import time as _time
            _time.sleep(2.0)

    return _unshard([res.results[c]["o"] for c in range(N_CORES)])


# revision 8
# speedup vs baseline: 1.0924x; 1.0873x over previous
"""Varlen causal GQA attention (B=4, S=1024, HQ=32, HK=8, D=128, fp32)
on 8 Trainium2 NeuronCores.

Sharding: tensor-parallel over the 8 kv heads (GQA groups stay together):
core i gets kv head i and query heads [4i, 4i+4), all 4 sequences. No
collectives; gather = concat along the head axis on host.

v3b: fp16 wire dtypes, one DMA per tensor per (seq | pair). Scores^T
(k x q) stream through nine (128,2,512) PSUM bins ordered so PV
chain-groups become ready as early as possible: bins 1-2 are the eight
causal-diagonal 128-wide pieces (one DVE [tri x4] mask mul per bin
after exp), bins 3-9 cover the off-diagonal in ascending-qi order.
PV chain-pair groups {0,1,2} / {3,4,5} / {6,7} are emitted as soon as
their last bin's exp lands, so the per-iteration tail (plain For_i
barriers every repeat iteration) is just chain 7 + normalize + store.
PV: P^T stationary with [V|1] moving (col 128 = sum exp), three
128-query chains share a PSUM bank; DVE reciprocal+mul normalizes into
fp16 o_t; one store per (b,h).
"""

import numpy as np
import ml_dtypes

import concourse.bass as bass
import concourse.tile as tile
import concourse.mybir as mybir
from concourse import bacc
from concourse.bass_utils import run_bass_kernel_spmd

B, S, D = 4, 1024, 128
HQ, HK = 32, 8
G = HQ // HK          # query heads per kv head (= per core)
N_CORES = 8
SCALE = 1.0 / float(np.sqrt(D))
KTW = 128             # key-tile width
KT = S // KTW         # key tiles per sequence
NQI = S // 128        # 128-query blocks per sequence

F32 = mybir.dt.float32
FP16 = mybir.dt.float16

# Nine 512-col score bins, (kt, c0, w) pieces, readiness-ordered.
# tri=True pieces get the causal-triangle mask after exp.
_DIAG_A = [(kt, kt * KTW, KTW, True) for kt in range(4)]
_DIAG_B = [(kt, kt * KTW, KTW, True) for kt in range(4, 8)]
SCORE_BINS = [
    _DIAG_A,                                             # b1
    _DIAG_B,                                             # b2
    [(0, 128, 384, False), (1, 256, 128, False)],        # b3
    [(1, 384, 128, False), (2, 384, 128, False),
     (0, 512, 256, False)],                              # b4
    [(1, 512, 256, False), (2, 512, 256, False)],        # b5
    [(3, 512, 256, False), (4, 640, 128, False),
     (0, 768, 128, False)],                              # b6
    [(1, 768, 128, False), (2, 768, 128, False),
     (3, 768, 128, False), (4, 768, 128, False)],        # b7
    [(5, 768, 128, False), (0, 896, 128, False),
     (1, 896, 128, False), (2, 896, 128, False)],        # b8
    [(3, 896, 128, False), (4, 896, 128, False),
     (5, 896, 128, False), (6, 896, 128, False)],        # b9
]
# PV chain groups (sharing one PSUM bank) -> bin index enabling them
PV_GROUPS = [([0, 1, 2], 2), ([3], 3), ([4, 5], 5), ([6], 7), ([7], 8)]
# normalize groups: chains sharing one po tile (slot -> chain)
PO_OF_CHAIN = {0: (0, 0), 1: (0, 1), 2: (0, 2),
               3: (1, 0), 4: (1, 1), 5: (1, 2),
               6: (2, 0), 7: (2, 1)}
PO_CHAINS = {0: [0, 1, 2], 1: [3, 4, 5], 2: [6, 7]}


def _check_bins():
    cov = {}
    for bi, pieces in enumerate(SCORE_BINS):
        off = 0
        for (kt, c0, w, tri) in pieces:
            for qi in range(c0 // KTW, (c0 + w) // KTW):
                assert (kt, qi) not in cov
                cov[(kt, qi)] = bi
            off += w
        assert off <= 512
    want = {(kt, qi) for qi in range(NQI) for kt in range(qi + 1)}
    assert set(cov) == want
    for chains, rb in PV_GROUPS:
        for qi in chains:
            for kt in range(qi + 1):
                assert cov[(kt, qi)] <= rb, (qi, kt, cov[(kt, qi)], rb)


_check_bins()


def build_nc(repeat: int = 1, ablate: str = "", mixed_tiles: bool = True):
    """Build the single-core Bass program (SPMD across 8 cores).

    repeat > 1 wraps the body in a hardware loop - used only for timing
    (marginal wall time per iteration approximates HW kernel time).
    mixed_tiles: pack bin pairs (b1,b2),(b4,b5),(b7,b8) into 4-bank
    (128,2,1024) PSUM tiles so ScalarE runs 6 exps/pair instead of 9.
    ablate: timing-only variants with reduced work (WRONG results):
      "pv" = halve PV chains; "dve" = skip mask/normalize.
    """
    nc = bacc.Bacc(None, target_bir_lowering=False, debug=False)

    qT = nc.dram_tensor("qT", [G // 2, B, D, 2, S], FP16, kind="ExternalInput")
    kT = nc.dram_tensor("kT", [B, D, S], FP16, kind="ExternalInput")
    v = nc.dram_tensor("v", [B, 128, KT, D], FP16, kind="ExternalInput")
    mk = nc.dram_tensor("mk", [D, 1024], FP16, kind="ExternalInput")
    o = nc.dram_tensor("o", [B, G, 128, NQI, D], FP16, kind="ExternalOutput")

    if mixed_tiles:
        # (use_big, [(bin_idx, base_col)]) per PSUM tile / exp
        TILE_PLAN = [(True, [(0, 0), (1, 512)]), (False, [(2, 0)]),
                     (True, [(3, 0), (4, 512)]), (False, [(5, 0)]),
                     (True, [(6, 0), (7, 512)]), (False, [(8, 0)])]
    else:
        TILE_PLAN = [(False, [(i, 0)]) for i in range(len(SCORE_BINS))]

    with tile.TileContext(nc) as tc:
        with (
            tc.tile_pool(name="cpool", bufs=1) as cpool,
            tc.tile_pool(name="kpool", bufs=2) as kpool,
            tc.tile_pool(name="vpool", bufs=2) as vpool,
            tc.tile_pool(name="qpool", bufs=2) as qpool,
            tc.tile_pool(name="pbig", bufs=5) as pbig,
            tc.tile_pool(name="psml", bufs=8) as psml,
            tc.tile_pool(name="opool", bufs=4) as opool,
            tc.tile_pool(name="rpool", bufs=8) as rpool,
            tc.tile_pool(name="psp4", bufs=1 if mixed_tiles else 0,
                         space="PSUM") as psp4,
            tc.tile_pool(name="psp2", bufs=1 if mixed_tiles else 3,
                         space="PSUM") as psp2,
            tc.tile_pool(name="ps_o", bufs=2, space="PSUM") as ps_o,
        ):
            # [tri x8]: mk[kk, 128a+q] = 1 iff q >= kk
            mask_t = cpool.tile([128, 1024], FP16)
            nc.sync.dma_start(out=mask_t[:], in_=mk[:])

            def emit_pv_chain(qi, q2, po, piece_map, v_t, hh):
                kts = list(range(qi + 1))
                if ablate == "pv":
                    kts = kts[:len(kts) // 2 + 1]
                for kt in kts:
                    pt, lo = piece_map[(kt, qi)]
                    nc.tensor.matmul(
                        po[:, q2, :],
                        lhsT=pt[:, hh, qi * KTW - lo:(qi + 1) * KTW - lo],
                        rhs=v_t[:, kt, :],
                        start=(kt == kts[0]),
                        stop=(kt == kts[-1]),
                    )

            def emit_pair(q_t, kt_t, v_t, o_ts, h0, b):
                piece_map = {}
                po_tiles = {}   # (hh, po_idx) -> tile
                for bi, pieces in enumerate(SCORE_BINS):
                    ps = psp.tile([128, 2, 512], F32, tag="ps", name="ps")
                    pt = ppool.tile([128, 2, 512], FP16, tag="pt", name="pt")
                    off = 0
                    for (kt, c0, w, tri) in pieces:
                        for hh in range(2):
                            nc.tensor.matmul(
                                ps[:, hh, off:off + w],
                                lhsT=kt_t[:, kt * KTW:(kt + 1) * KTW],
                                rhs=q_t[:, hh, c0:c0 + w],
                                start=True, stop=True,
                            )
                        lo = c0 - off
                        for qi in range(c0 // KTW, (c0 + w) // KTW):
                            piece_map[(kt, qi)] = (pt, lo)
                        off += w
                    nc.scalar.activation(
                        pt[:, :, 0:off], ps[:, :, 0:off],
                        mybir.ActivationFunctionType.Exp, scale=SCALE,
                    )
                    if pieces[0][3] and ablate != "dve":
                        # diagonal bin: one mul masks all four triangles
                        nc.vector.tensor_mul(
                            pt[:, :, 0:off], pt[:, :, 0:off],
                            mask_t[:, None, 0:off].broadcast_to([128, 2, off]))
                    for chains, rb in PV_GROUPS:
                        if rb != bi:
                            continue
                        for qi in chains:
                            pidx, slot = PO_OF_CHAIN[qi]
                            for hh in range(2):
                                key = (hh, pidx)
                                if key not in po_tiles:
                                    po_tiles[key] = ps_o.tile(
                                        [128, 3, KTW + 1], F32, tag="po",
                                        name="po")
                                emit_pv_chain(qi, slot, po_tiles[key],
                                              piece_map, v_t, hh)
                            # normalize once the po group is complete
                            if qi == PO_CHAINS[pidx][-1]:
                                nsl = len(PO_CHAINS[pidx])
                                q0 = PO_CHAINS[pidx][0]
                                for hh in range(2):
                                    po = po_tiles[(hh, pidx)]
                                    if ablate == "dve":
                                        nc.vector.tensor_copy(
                                            o_ts[hh][:, q0, :],
                                            po[:, 0, 0:KTW])
                                        continue
                                    rec = rpool.tile([128, 3], F32, tag="rec",
                                                     name="rec")
                                    nc.vector.reciprocal(
                                        rec[:, 0:nsl], po[:, 0:nsl, KTW])
                                    nc.vector.tensor_mul(
                                        o_ts[hh][:, q0:q0 + nsl, :],
                                        po[:, 0:nsl, 0:KTW],
                                        rec[:, 0:nsl, None].broadcast_to(
                                            [128, nsl, KTW]),
                                    )
                for hh in range(2):
                    nc.gpsimd.dma_start(out=o[b, h0 + hh], in_=o_ts[hh][:])

            def body(_iv=None):
                for b in range(B):
                    kt_t = kpool.tile([128, S], FP16, tag="kt", name="kt_t")
                    nc.sync.dma_start(out=kt_t[:, 0:512], in_=kT[b][:, 0:512])
                    nc.sync.dma_start(out=kt_t[:, 512:S], in_=kT[b][:, 512:S])
                    v_t = vpool.tile([128, KT, KTW + 1], FP16, tag="vt",
                                     name="v_t")
                    nc.sync.dma_start(out=v_t[:, :, 0:KTW], in_=v[b])
                    nc.vector.memset(v_t[:, :, KTW:KTW + 1], 1.0)
                    for hp in range(G // 2):
                        h0 = hp * 2
                        q_t = qpool.tile([128, 2, S], FP16, tag="qt",
                                         name="q_t")
                        nc.gpsimd.dma_start(out=q_t[:, :, 0:512],
                                            in_=qT[hp, b][:, :, 0:512])
                        nc.gpsimd.dma_start(out=q_t[:, :, 512:S],
                                            in_=qT[hp, b][:, :, 512:S])
                        o_ts = [opool.tile([128, NQI, KTW], FP16, tag="ot",
                                           name="o_t") for _ in range(2)]
                        emit_pair(q_t, kt_t, v_t, o_ts, h0, b)

            if repeat == 1:
                body()
            else:
                with tc.For_i(0, repeat, 1) as iv:
                    body(iv)

    nc.compile()
    return nc


def _build_mask() -> np.ndarray:
    """[tri x4]: mk[kk, 128a+q] = 1 iff q >= kk."""
    kk = np.arange(128)[:, None]
    qq = np.arange(128)[None, :]
    tri = (qq >= kk).astype(np.float16)
    return np.tile(tri, (1, 4))


def _core_inputs(q: np.ndarray, k: np.ndarray, v: np.ndarray):
    """Slice + lay out per-core inputs. Host-side shard/layout step."""
    mask = _build_mask()
    q5 = q.reshape(B, S, HK, G, D)
    k4 = k.reshape(B, S, HK, D)
    v4 = v.reshape(B, S, HK, D)
    in_maps = []
    for c in range(N_CORES):
        qt = q5[:, :, c, :, :].transpose(2, 0, 3, 1)          # (G,B,D,S)
        qT = np.ascontiguousarray(
            qt.reshape(G // 2, 2, B, D, S).transpose(0, 2, 3, 1, 4)
        ).astype(np.float16)                                   # (G/2,B,D,2,S)
        kT = np.ascontiguousarray(
            k4[:, :, c, :].transpose(0, 2, 1)).astype(np.float16)  # (B,D,S)
        vb = np.ascontiguousarray(
            v4[:, :, c, :].reshape(B, KT, 128, D).transpose(0, 2, 1, 3)
        ).astype(np.float16)                                   # (B,128,KT,D)
        in_maps.append({"qT": qT, "kT": kT, "v": vb, "mk": mask})
    return in_maps


def _unshard(core_outs) -> np.ndarray:
    """core_outs[c]: (B, G, 128, NQI, D) fp16 -> (B*S, HQ, D) fp32."""
    out = np.empty((B, S, HQ, D), np.float32)
    for c, oc in enumerate(core_outs):
        ob = np.asarray(oc, dtype=np.float32).transpose(0, 3, 2, 1, 4)
        out[:, :, c * G:(c + 1) * G, :] = ob.reshape(B, S, G, D)
    return out.reshape(B * S, HQ, D)


_NC_CACHE = {}


def kernel(q, k, v, cu_seqlens_q=None, cu_seqlens_k=None,
           max_seqlen_q=None, max_seqlen_k=None) -> np.ndarray:
    q = np.asarray(q, dtype=np.float32)
    k = np.asarray(k, dtype=np.float32)
    v = np.asarray(v, dtype=np.float32)
    assert q.shape == (B * S, HQ, D) and k.shape == (B * S, HK, D)

    if "nc" not in _NC_CACHE:
        _NC_CACHE["nc"] = build_nc(repeat=1)
    nc = _NC_CACHE["nc"]

    in_maps = _core_inputs(q, k, v)
    res = None
    for attempt in range(3):
        try:
            res = run_bass_kernel_spmd(nc, in_maps,
                                       core_ids=list(range(N_CORES)))
            break
        except Exception:
            # a wedged NeuronCore fails once and resets; retry clean
            if attempt == 2:
                raise
            import time as _time
            _time.sleep(2.0)

    return _unshard([res.results[c]["o"] for c in range(N_CORES)])


# revision 21
# speedup vs baseline: 1.3710x; 1.2550x over previous
"""Varlen causal GQA attention (B=4, S=1024, HQ=32, HK=8, D=128, fp32)
on 8 Trainium2 NeuronCores.

Sharding: tensor-parallel over the 8 kv heads (GQA groups stay together):
core i gets kv head i and query heads [4i, 4i+4), all 4 sequences. No
collectives; gather = concat along the head axis on host.

v3b: fp16 wire dtypes, one DMA per tensor per (seq | pair). Scores^T
(k x q) stream through nine (128,2,512) PSUM bins ordered so PV
chain-groups become ready as early as possible: bins 1-2 are the eight
causal-diagonal 128-wide pieces (one DVE [tri x4] mask mul per bin
after exp), bins 3-9 cover the off-diagonal in ascending-qi order.
PV chain-pair groups {0,1,2} / {3,4,5} / {6,7} are emitted as soon as
their last bin's exp lands, so the per-iteration tail (plain For_i
barriers every repeat iteration) is just chain 7 + normalize + store.
PV: P^T stationary with [V|1] moving (col 128 = sum exp), three
128-query chains share a PSUM bank; DVE reciprocal+mul normalizes into
fp16 o_t; one store per (b,h).
"""

import numpy as np
import ml_dtypes

import concourse.bass as bass
import concourse.tile as tile
import concourse.mybir as mybir
from concourse import bacc
from concourse.bass_utils import run_bass_kernel_spmd

B, S, D = 4, 1024, 128
HQ, HK = 32, 8
G = HQ // HK          # query heads per kv head (= per core)
N_CORES = 8
SCALE = 1.0 / float(np.sqrt(D))
KTW = 128             # key-tile width
KT = S // KTW         # key tiles per sequence
NQI = S // 128        # 128-query blocks per sequence

F32 = mybir.dt.float32
FP16 = mybir.dt.float16

# Score bins: lists of (kt, c0, w, tri) pieces laid out back-to-back in
# one PSUM tile, readiness-ordered so PV chain groups unblock early.
# tri=True pieces get the causal-triangle mask after exp. No piece
# crosses a 512-col PSUM bank boundary inside its tile.
_T, _F = True, False
BINS_768 = [
    [(0, 0, 128, _T), (1, 128, 128, _T), (2, 256, 128, _T),
     (3, 384, 128, _T), (4, 512, 128, _T), (5, 640, 128, _T)],
    [(0, 128, 384, _F), (6, 768, 128, _T), (7, 896, 128, _T),
     (1, 256, 128, _F)],
    [(1, 384, 128, _F), (2, 384, 128, _F), (0, 512, 256, _F),
     (1, 512, 256, _F)],
    [(2, 512, 256, _F), (3, 512, 256, _F), (4, 640, 128, _F),
     (0, 768, 128, _F)],
    [(1, 768, 128, _F), (2, 768, 128, _F), (3, 768, 128, _F),
     (4, 768, 128, _F), (5, 768, 128, _F), (0, 896, 128, _F)],
    [(1, 896, 128, _F), (2, 896, 128, _F), (3, 896, 128, _F),
     (4, 896, 128, _F), (5, 896, 128, _F), (6, 896, 128, _F)],
]
PV_GROUPS_768 = [([0, 1, 2], 1), ([3], 2), ([4, 5], 3), ([6], 4), ([7], 5)]

BINS_512 = [
    [(kt, kt * KTW, KTW, _T) for kt in range(4)],
    [(kt, kt * KTW, KTW, _T) for kt in range(4, 8)],
    [(0, 128, 384, _F), (1, 256, 128, _F)],
    [(1, 384, 128, _F), (2, 384, 128, _F), (0, 512, 256, _F)],
    [(1, 512, 256, _F), (2, 512, 256, _F)],
    [(3, 512, 256, _F), (4, 640, 128, _F), (0, 768, 128, _F)],
    [(1, 768, 128, _F), (2, 768, 128, _F), (3, 768, 128, _F),
     (4, 768, 128, _F)],
    [(5, 768, 128, _F), (0, 896, 128, _F), (1, 896, 128, _F),
     (2, 896, 128, _F)],
    [(3, 896, 128, _F), (4, 896, 128, _F), (5, 896, 128, _F),
     (6, 896, 128, _F)],
]
PV_GROUPS_512 = [([0, 1, 2], 2), ([3], 3), ([4, 5], 5), ([6], 7), ([7], 8)]

# normalize groups: chains sharing one po tile (chain -> (tile, slot))
PO_OF_CHAIN = {0: (0, 0), 1: (0, 1), 2: (0, 2),
               3: (1, 0), 4: (1, 1), 5: (1, 2),
               6: (2, 0), 7: (2, 1)}
PO_CHAINS = {0: [0, 1, 2], 1: [3, 4, 5], 2: [6, 7]}


def _check_bins(score_bins, pv_groups, tile_w):
    cov = {}
    for bi, pieces in enumerate(score_bins):
        off = 0
        for (kt, c0, w, tri) in pieces:
            assert off // 512 == (off + w - 1) // 512, (bi, off, w)
            for qi in range(c0 // KTW, (c0 + w) // KTW):
                assert (kt, qi) not in cov
                cov[(kt, qi)] = bi
            off += w
        assert off <= tile_w
    want = {(kt, qi) for qi in range(NQI) for kt in range(qi + 1)}
    assert set(cov) == want
    for chains, rb in pv_groups:
        for qi in chains:
            for kt in range(qi + 1):
                assert cov[(kt, qi)] <= rb, (qi, kt, cov[(kt, qi)], rb)


_check_bins(BINS_768, PV_GROUPS_768, 768)
_check_bins(BINS_512, PV_GROUPS_512, 512)


def build_nc(repeat: int = 1, ablate: str = "", tile_w: int = 512):
    """Build the single-core Bass program (SPMD across 8 cores).

    repeat > 1 wraps the body in a hardware loop - used only for timing
    (marginal wall time per iteration approximates HW kernel time).
    tile_w: 512 = nine 2-bank score tiles (9 exps/pair, triple-
    buffered). (768 tiles are illegal: with two heads interleaved the
    head-1 base lands mid-bank and matmul outputs may not cross a PSUM
    bank boundary.)
    ablate: timing-only variants with reduced work (WRONG results):
      "pv" = halve PV chains; "dve" = skip mask/normalize.
    """
    nc = bacc.Bacc(None, target_bir_lowering=False, debug=False)

    qT = nc.dram_tensor("qT", [G // 2, B, D, 2, S], FP16, kind="ExternalInput")
    kT = nc.dram_tensor("kT", [B, D, S], FP16, kind="ExternalInput")
    v = nc.dram_tensor("v", [B, 128, KT, D], FP16, kind="ExternalInput")
    mk = nc.dram_tensor("mk", [D, 1024], FP16, kind="ExternalInput")
    o = nc.dram_tensor("o", [B, G, 128, NQI, D], FP16, kind="ExternalOutput")

    score_bins = BINS_768 if tile_w == 768 else BINS_512
    pv_groups = PV_GROUPS_768 if tile_w == 768 else PV_GROUPS_512
    n_pt = 2 * len(score_bins) + 2   # pt tiles: ~2 pairs live + slack

    with tile.TileContext(nc) as tc:
        with (
            tc.tile_pool(name="cpool", bufs=1) as cpool,
            tc.tile_pool(name="kpool", bufs=2) as kpool,
            tc.tile_pool(name="vpool", bufs=2) as vpool,
            tc.tile_pool(name="qpool", bufs=2) as qpool,
            tc.tile_pool(name="ppool", bufs=n_pt) as ppool,
            tc.tile_pool(name="opool", bufs=4) as opool,
            tc.tile_pool(name="rpool", bufs=8) as rpool,
            tc.tile_pool(name="psp", bufs=2 if tile_w == 768 else 3,
                         space="PSUM") as psp,
            tc.tile_pool(name="ps_o", bufs=2, space="PSUM") as ps_o,
        ):
            # [tri x8]: mk[kk, 128a+q] = 1 iff q >= kk
            mask_t = cpool.tile([128, 1024], FP16)
            nc.sync.dma_start(out=mask_t[:], in_=mk[:])

            def emit_pv_chain(qi, q2, po, piece_map, v_t, hh):
                kts = list(range(qi + 1))
                if ablate == "pv":
                    kts = kts[:len(kts) // 2 + 1]
                for kt in kts:
                    pt, lo = piece_map[(kt, qi)]
                    nc.tensor.matmul(
                        po[:, q2, :],
                        lhsT=pt[:, hh, qi * KTW - lo:(qi + 1) * KTW - lo],
                        rhs=v_t[:, kt, :],
                        start=(kt == kts[0]),
                        stop=(kt == kts[-1]),
                    )

            def emit_pair(q_t, kt_t, v_t, o_ts, h0, b):
                piece_map = {}
                po_tiles = {}   # (hh, po_idx) -> tile

                def emit_pv_for_bins(tile_bins):
                    for chains, rb in pv_groups:
                        if rb not in tile_bins:
                            continue
                        for qi in chains:
                            pidx, slot = PO_OF_CHAIN[qi]
                            for hh in range(2):
                                key = (hh, pidx)
                                if key not in po_tiles:
                                    po_tiles[key] = ps_o.tile(
                                        [128, 3, KTW + 1], F32, tag="po",
                                        name="po")
                                emit_pv_chain(qi, slot, po_tiles[key],
                                              piece_map, v_t, hh)
                            # normalize once the po group is complete
                            if qi == PO_CHAINS[pidx][-1]:
                                nsl = len(PO_CHAINS[pidx])
                                q0 = PO_CHAINS[pidx][0]
                                for hh in range(2):
                                    po = po_tiles[(hh, pidx)]
                                    if ablate == "dve":
                                        nc.vector.tensor_copy(
                                            o_ts[hh][:, q0, :],
                                            po[:, 0, 0:KTW])
                                        continue
                                    rec = rpool.tile([128, 3], F32, tag="rec",
                                                     name="rec")
                                    nc.vector.reciprocal(
                                        rec[:, 0:nsl], po[:, 0:nsl, KTW])
                                    nc.vector.tensor_mul(
                                        o_ts[hh][:, q0:q0 + nsl, :],
                                        po[:, 0:nsl, 0:KTW],
                                        rec[:, 0:nsl, None].broadcast_to(
                                            [128, nsl, KTW]),
                                    )

                pending_bins = set()
                for bi, pieces in enumerate(score_bins):
                    ps = psp.tile([128, 2, tile_w], F32, tag="ps", name="ps")
                    pt = ppool.tile([128, 2, tile_w], FP16, tag="pt",
                                    name="pt")
                    off = 0
                    tri_runs = []
                    for (kt, c0, w, tri) in pieces:
                        for hh in range(2):
                            nc.tensor.matmul(
                                ps[:, hh, off:off + w],
                                lhsT=kt_t[:, kt * KTW:(kt + 1) * KTW],
                                rhs=q_t[:, hh, c0:c0 + w],
                                start=True, stop=True,
                            )
                        lo = c0 - off
                        for qi in range(c0 // KTW, (c0 + w) // KTW):
                            piece_map[(kt, qi)] = (pt, lo)
                        if tri:
                            if tri_runs and tri_runs[-1][0] \
                                    + tri_runs[-1][1] == off:
                                tri_runs[-1][1] += w
                            else:
                                tri_runs.append([off, w])
                        off += w
                    nc.scalar.activation(
                        pt[:, :, 0:off], ps[:, :, 0:off],
                        mybir.ActivationFunctionType.Exp, scale=SCALE,
                    )
                    if ablate != "dve":
                        for (t0, tw) in tri_runs:
                            nc.vector.tensor_mul(
                                pt[:, :, t0:t0 + tw], pt[:, :, t0:t0 + tw],
                                mask_t[:, None, 0:tw].broadcast_to(
                                    [128, 2, tw]))
                    emit_pv_for_bins(pending_bins)
                    pending_bins = {bi}
                emit_pv_for_bins(pending_bins)
                for hh in range(2):
                    nc.gpsimd.dma_start(out=o[b, h0 + hh], in_=o_ts[hh][:])

            def body(_iv=None):
                for b in range(B):
                    kt_t = kpool.tile([128, S], FP16, tag="kt", name="kt_t")
                    nc.sync.dma_start(out=kt_t[:, 0:512], in_=kT[b][:, 0:512])
                    nc.sync.dma_start(out=kt_t[:, 512:S], in_=kT[b][:, 512:S])
                    v_t = vpool.tile([128, KT, KTW + 1], FP16, tag="vt",
                                     name="v_t")
                    nc.sync.dma_start(out=v_t[:, :, 0:KTW], in_=v[b])
                    nc.vector.memset(v_t[:, :, KTW:KTW + 1], 1.0)
                    for hp in range(G // 2):
                        h0 = hp * 2
                        q_t = qpool.tile([128, 2, S], FP16, tag="qt",
                                         name="q_t")
                        nc.gpsimd.dma_start(out=q_t[:, :, 0:512],
                                            in_=qT[hp, b][:, :, 0:512])
                        nc.gpsimd.dma_start(out=q_t[:, :, 512:S],
                                            in_=qT[hp, b][:, :, 512:S])
                        o_ts = [opool.tile([128, NQI, KTW], FP16, tag="ot",
                                           name="o_t") for _ in range(2)]
                        emit_pair(q_t, kt_t, v_t, o_ts, h0, b)

            if repeat == 1:
                body()
            else:
                with tc.For_i(0, repeat, 1) as iv:
                    body(iv)

    nc.compile()
    return nc


def _build_mask() -> np.ndarray:
    """[tri x8]: mk[kk, 128a+q] = 1 iff q >= kk."""
    kk = np.arange(128)[:, None]
    qq = np.arange(128)[None, :]
    tri = (qq >= kk).astype(np.float16)
    return np.tile(tri, (1, 8))


def _core_inputs(q: np.ndarray, k: np.ndarray, v: np.ndarray):
    """Slice + lay out per-core inputs. Host-side shard/layout step."""
    mask = _build_mask()
    q5 = q.reshape(B, S, HK, G, D)
    k4 = k.reshape(B, S, HK, D)
    v4 = v.reshape(B, S, HK, D)
    in_maps = []
    for c in range(N_CORES):
        qt = q5[:, :, c, :, :].transpose(2, 0, 3, 1)          # (G,B,D,S)
        qT = np.ascontiguousarray(
            qt.reshape(G // 2, 2, B, D, S).transpose(0, 2, 3, 1, 4)
        ).astype(np.float16)                                   # (G/2,B,D,2,S)
        kT = np.ascontiguousarray(
            k4[:, :, c, :].transpose(0, 2, 1)).astype(np.float16)  # (B,D,S)
        vb = np.ascontiguousarray(
            v4[:, :, c, :].reshape(B, KT, 128, D).transpose(0, 2, 1, 3)
        ).astype(np.float16)                                   # (B,128,KT,D)
        in_maps.append({"qT": qT, "kT": kT, "v": vb, "mk": mask})
    return in_maps


def _unshard(core_outs) -> np.ndarray:
    """core_outs[c]: (B, G, 128, NQI, D) fp16 -> (B*S, HQ, D) fp32."""
    out = np.empty((B, S, HQ, D), np.float32)
    for c, oc in enumerate(core_outs):
        ob = np.asarray(oc, dtype=np.float32).transpose(0, 3, 2, 1, 4)
        out[:, :, c * G:(c + 1) * G, :] = ob.reshape(B, S, G, D)
    return out.reshape(B * S, HQ, D)


_NC_CACHE = {}


def kernel(q, k, v, cu_seqlens_q=None, cu_seqlens_k=None,
           max_seqlen_q=None, max_seqlen_k=None) -> np.ndarray:
    q = np.asarray(q, dtype=np.float32)
    k = np.asarray(k, dtype=np.float32)
    v = np.asarray(v, dtype=np.float32)
    assert q.shape == (B * S, HQ, D) and k.shape == (B * S, HK, D)

    if "nc" not in _NC_CACHE:
        _NC_CACHE["nc"] = build_nc(repeat=1)
    nc = _NC_CACHE["nc"]

    in_maps = _core_inputs(q, k, v)
    res = None
    for attempt in range(3):
        try:
            res = run_bass_kernel_spmd(nc, in_maps,
                                       core_ids=list(range(N_CORES)))
            break
        except Exception:
            # a wedged NeuronCore fails once and resets; retry clean
            if attempt == 2:
                raise
            import time as _time
            _time.sleep(2.0)

    return _unshard([res.results[c]["o"] for c in range(N_CORES)])
